# revision 41
# baseline (speedup 1.0000x reference)
"""Distributed causal multi-head attention forward for one TRN2 chip (8 NeuronCores).

Problem (nn_Attention): B=2, S=2048, d_model=1024, 16 heads x 64.
    attn_in = x + pos_embed
    q = attn_in @ W_Q + b_Q ; k = attn_in @ W_K + b_K ; v = x @ W_V + b_V
    out = softmax(causal(q k^T / sqrt(64))) v @ W_O + b_O

Sharding: data-parallel over batch (2 groups of 4 cores), tensor-parallel over
heads inside each group (4 heads per core).  Each core computes z = attn(v)
for its 4 heads over the full sequence; the normalized z^T is staged to DRAM
in bf16 and exchanged with ONE AllGather per rep (bypass op - collectives
have a ~35us fixed cost on this stack and bf16 *reducing* collectives hit a
~100us software-add path, so a single gather of the small z tensor beats any
ReduceScatter of W_O partials).  Every core then extracts its own 128-row
stripe of each 512-row block from the gathered z with a selection-matrix
matmul (a per-core 0/1 input - keeps the program SPMD-identical across
cores), applies the FULL W_O to just those rows, and writes f32 output
stripes.  The host reassembles [B, S, D] by pure indexing.

Layout/perf notes:
  * All inputs arrive in bf16 (host-side cast): halves HBM traffic vs fp32;
    all matmuls accumulate in fp32 PSUM (~0.9% final error vs the 2e-2
    gate).  Weights come host-pre-chunked so each weight tensor is ONE
    contiguous [128, ...] DMA; descriptor generation costs ~625ns per DMA
    instruction on the DGE queues, so DMA count matters as much as bytes.
    DMAs are split across both DGE queues (SP + Activation).
  * x/pos are loaded as full-sequence rows ([128, 2048] per d_model chunk).
  * Scores are built transposed (keys on partitions) so softmax-exp feeds
    P@V directly.  Matmul cost is (out free size) cycles, so P@V runs
    "z^T-wise": out [q=128, d_head+1] with a 65-wide moving dim - half the
    cycles of the [65, q] orientation.
  * Softmax denominator: ones-column appended to V; normalization is a
    per-partition reciprocal + tensor_scalar on DVE (queries on partitions),
    zero PE cycles.  GPSIMD/Pool cannot read PSUM, so all PSUM->SBUF copies
    live on DVE; Pool only does the x+pos adds (rs modes) or nothing (ag).
  * Emission is software-pipelined and CIRCULAR across reps: B(J) injects
    the QKV projections of block J+1 (wrapping into the next rep with
    anti-dependency-safe pair scheduling) and the W_O of block J of the
    previous rep between score chunks, so the in-order PE queue stays fed
    while the Activation engine (exp, the #2 busiest engine) catches up.
  * The rs_* build flags keep the earlier ReduceScatter variants for A/B.
"""

import math

import numpy as np

import concourse.bass as bass  # noqa: F401  (bass must import before bacc)
import concourse.mybir as mybir
from concourse import bacc, tile
from concourse.bass_utils import run_bass_kernel_spmd

B, S, D = 2, 2048, 1024
NH, DH = 16, 64
N_CORES = 8
GPC = 4                      # cores per batch group
HPC = NH // GPC              # heads per core
QB = 512                     # query-block rows
NJ = S // QB                 # query blocks
KCH = 128                    # key chunk (= row tile)
DCH = D // 128               # d_model chunks
RG = [[0, 1, 2, 3], [4, 5, 6, 7]]
SCALE = 1.0 / float(np.sqrt(DH))

F32 = mybir.dt.float32


class _ActCopy:
    """Adapter: .tensor_copy on the Activation engine (activation Copy)."""
    def __init__(self, nc):
        self._nc = nc

    def tensor_copy(self, out, in_):
        self._nc.scalar.copy(out, in_)
BF16 = mybir.dt.bfloat16
EXP = mybir.ActivationFunctionType.Exp
ADD = mybir.AluOpType.add
MUL = mybir.AluOpType.mult


def build_nc(reps: int = 1, collective: bool = True, bias: bool = True,
             rs_f32: bool = False, rs_rep: bool = False, ag: bool = False):
    """Build the per-core Bass graph.  `reps` repeats the whole computation
    (used only for wall-clock timing calibration; grading uses reps=1)."""
    nc = bacc.Bacc("TRN2", target_bir_lowering=False, debug=False,
                   num_devices=N_CORES)

    xT = nc.dram_tensor("xT", [D, S], BF16, kind="ExternalInput").ap()
    posT = nc.dram_tensor("posT", [D, S], BF16, kind="ExternalInput").ap()
    # host-pre-chunked: [128, kc, 256] flattened
    wqp = nc.dram_tensor("wqp", [128, DCH * HPC * DH], BF16,
                         kind="ExternalInput").ap()
    wkp = nc.dram_tensor("wkp", [128, DCH * HPC * DH], BF16,
                         kind="ExternalInput").ap()
    wvp = nc.dram_tensor("wvp", [128, DCH * HPC * DH], BF16,
                         kind="ExternalInput").ap()
    if ag:
        wof = nc.dram_tensor("wof", [128, 8 * D], BF16,
                             kind="ExternalInput").ap()
        sel = nc.dram_tensor("sel", [128, GPC * 128], BF16,
                             kind="ExternalInput").ap()
    else:
        wop = nc.dram_tensor("wop", [128, 2 * D], BF16,
                             kind="ExternalInput").ap()
    bqT = nc.dram_tensor("bqT", [KCH, 2], F32, kind="ExternalInput").ap()
    bkT = nc.dram_tensor("bkT", [KCH, 2], F32, kind="ExternalInput").ap()
    bv = nc.dram_tensor("bv", [1, HPC * DH], BF16, kind="ExternalInput").ap()
    bo = nc.dram_tensor("bo", [1, D], BF16, kind="ExternalInput").ap()
    masks = nc.dram_tensor("masks", [KCH, 2 * KCH], BF16,
                           kind="ExternalInput").ap()
    rdt = F32 if rs_f32 else BF16
    if ag:
        rdt = F32
    out_ext = nc.dram_tensor("out", [S // GPC, D], rdt,
                             kind="ExternalOutput").ap()

    act_copy = _ActCopy(nc)
    with tile.TileContext(nc) as tc:
        with tc.tile_pool(name="wp", bufs=1) as wp, \
             tc.tile_pool(name="qkv", bufs=1) as qp, \
             tc.tile_pool(name="xfp", bufs=8) as xfp, \
             tc.tile_pool(name="posp", bufs=2) as posp, \
             tc.tile_pool(name="xpp", bufs=8) as xpp, \
             tc.tile_pool(name="p2p", bufs=16) as p2p, \
             tc.tile_pool(name="rsp", bufs=4) as rsp, \
             tc.tile_pool(name="ztsb", bufs=16) as ztsbp, \
             tc.tile_pool(name="ztp", bufs=16) as ztpp, \
             tc.tile_pool(name="osb", bufs=2) as osbp, \
             tc.tile_pool(name="zgp", bufs=1) as zgp, \
             tc.tile_pool(name="psS", bufs=2, space="PSUM") as psS, \
             tc.tile_pool(name="psZT", bufs=2, space="PSUM") as psZT, \
             tc.tile_pool(name="psA", bufs=2, space="PSUM") as psA, \
             tc.tile_pool(name="dram", bufs=2, space="DRAM") as dp:

            # ---------- persistent weight tiles ----------
            wq_t = wp.tile([128, DCH, HPC * DH], BF16, tag="wq")
            wk_t = wp.tile([128, DCH, HPC * DH], BF16, tag="wk")
            wv_t = wp.tile([128, DCH, HPC * DH], BF16, tag="wv")
            if ag:
                wo_t = wp.tile([128, 8, D], BF16, tag="wo")
                sel_t = wp.tile([128, GPC, 128], BF16, tag="sel")
            else:
                wo_t = wp.tile([128, 2, D], BF16, tag="wo")
            tri_m = wp.tile([KCH, 2, KCH], BF16, tag="tri_m")
            bqT_t = wp.tile([KCH, 2], F32, tag="bqT")
            bkT_t = wp.tile([KCH, 2], F32, tag="bkT")
            bv_t = wp.tile([1, HPC * DH], BF16, tag="bv")
            bo_t = wp.tile([1, D], BF16, tag="bo")

            def emit_weight_dmas():
                # wq first (first consumer); split across the two DGE queues
                nc.sync.dma_start(wq_t[:], wqp[:, :])
                nc.scalar.dma_start(wk_t[:], wkp[:, :])
                nc.sync.dma_start(wv_t[:], wvp[:, :])
                if ag:
                    nc.scalar.dma_start(wo_t[:], wof[:, :])
                    nc.scalar.dma_start(sel_t[:], sel[:, :])
                else:
                    nc.scalar.dma_start(wo_t[:], wop[:, :])
                nc.scalar.dma_start(tri_m[:, :, :], masks[:, :])
                nc.scalar.dma_start(bqT_t[:], bqT[:, :])
                nc.scalar.dma_start(bkT_t[:], bkT[:, :])
                nc.scalar.dma_start(bv_t[:], bv[:, :])
                nc.scalar.dma_start(bo_t[:], bo[:, :])

            if bias:
                emit_weight_dmas()
                ones = wp.tile([1, KCH], BF16, tag="ones")
                nc.vector.memset(ones[:], 1.0)
                bv_ps = psA.tile([128, HPC, DH], F32, tag="a_ps")
                nc.tensor.matmul(bv_ps[:], ones[0:1, :], bv_t[0:1, :],
                                 start=True, stop=True)
                bv_bc = wp.tile([128, HPC, DH], BF16, tag="bv_bc")
                nc.vector.tensor_copy(bv_bc[:], bv_ps[:])
                bo_bc = wp.tile([128, D], BF16, tag="bo_bc")
                for ms in range(2):
                    bo_ps = psA.tile([128, 512], F32, tag="a_ps")
                    nc.tensor.matmul(bo_ps[:], ones[0:1, :],
                                     bo_t[0:1, 512 * ms:512 * (ms + 1)],
                                     start=True, stop=True)
                    nc.vector.tensor_copy(
                        bo_bc[:, 512 * ms:512 * (ms + 1)], bo_ps[:])
            weights_loaded = bool(bias)

            # persistent per-rep activations, double-buffered by rep parity
            # so the next rep's QKV can overlap this rep's attention tail
            npar = 1
            qT_par, kT_par, va_par = [], [], []
            for par in range(npar):
                qT, kT = [], []
                for p in range(2):
                    t_q = qp.tile([128, S], BF16, tag=f"qT{par}{p}",
                                  name="t_q")
                    qT.append(t_q)
                    t_k = qp.tile([128, S], BF16, tag=f"kT{par}{p}",
                                  name="t_k")
                    kT.append(t_k)
                v_aug = []
                for rt in range(S // KCH):
                    t_v = qp.tile([128, HPC, DH + 1], BF16,
                                  tag=f"va{par}_{rt}", name="t_v")
                    nc.vector.memset(t_v[:, :, DH:DH + 1], 1.0)
                    v_aug.append(t_v)
                qT_par.append(qT)
                kT_par.append(kT)
                va_par.append(v_aug)

            # rolling per-block state (overwritten every rep)
            x_par = {}         # parity -> list of x row tiles
            xp_par = {}        # parity -> list of x+pos row tiles
            zts = {}           # (J, hp, qsub) -> zt_sb tile
            prt = [None] * NJ
            tz_stash = {}
            prtz = None
            zg_prev = None

            def emit_producers(par, defer_adds=False):
                """DMA full-sequence x/pos rows + x+pos adds for one rep.
                With defer_adds the DVE adds are returned as closures so the
                caller can weave them into the filler stream (emitting all 8
                up-front head-of-line-blocks the DVE queue behind DMAs)."""
                xs, xps, adds = [], [], []
                for kc in range(DCH):
                    ksl = slice(128 * kc, 128 * (kc + 1))
                    t_xc = xfp.tile([128, S], BF16, tag="xc", name="xc")
                    nc.sync.dma_start(t_xc[:], xT[ksl, :])
                    t_pos = posp.tile([128, S], BF16, tag="pos", name="pos")
                    nc.scalar.dma_start(t_pos[:], posT[ksl, :])
                    t_xp = xpp.tile([128, S], BF16, tag="xp", name="xp")
                    if defer_adds:
                        adds.append(lambda a=t_xp, b=t_xc, c=t_pos:
                                    nc.vector.tensor_add(a[:], b[:], c[:]))
                    else:
                        nc.vector.tensor_add(t_xp[:], t_xc[:], t_pos[:])
                    xs.append(t_xc)
                    xps.append(t_xp)
                x_par[par] = xs
                xp_par[par] = xps
                return adds

            def qk_group(J, par, dst, w_t, b_t, p, ceng=None):
                xp_t = xp_par[par]
                jsl = slice(QB * J, QB * (J + 1))
                psl = slice(128 * p, 128 * (p + 1))
                acc = psA.tile([128, QB], F32, tag="a_ps")
                for kc in range(DCH):
                    nc.tensor.matmul(acc[:], w_t[:, kc, psl],
                                     xp_t[kc][:, jsl],
                                     start=(kc == 0), stop=(kc == DCH - 1))
                if bias:
                    nc.vector.tensor_scalar(
                        dst[p][:, jsl], acc[:], b_t[:, p:p + 1], None, ADD)
                else:
                    (ceng or nc.vector).tensor_copy(dst[p][:, jsl], acc[:])

            def v_group(J, par, r, ceng=None):
                v_aug = va_par[par]
                x_t = x_par[par]
                rt = 4 * J + r
                rsl = slice(QB * J + 128 * r, QB * J + 128 * (r + 1))
                vacc = psA.tile([128, HPC, DH], F32, tag="a_ps")
                for kc in range(DCH):
                    nc.tensor.matmul(vacc[:], x_t[kc][:, rsl], wv_t[:, kc, :],
                                     start=(kc == 0), stop=(kc == DCH - 1))
                if bias:
                    nc.vector.tensor_tensor(
                        v_aug[rt][:, :, 0:DH], vacc[:], bv_bc[:], ADD)
                else:
                    (ceng or nc.vector).tensor_copy(
                        v_aug[rt][:, :, 0:DH], vacc[:])

            def qkv_groups(J, par, ceng=None):
                gs = []
                for dst, w_t, b_t in ((qT_par[par], wq_t, bqT_t),
                                      (kT_par[par], wk_t, bkT_t)):
                    for p in range(2):
                        gs.append(lambda J=J, par=par, dst=dst, w_t=w_t,
                                  b_t=b_t, p=p:
                                  qk_group(J, par, dst, w_t, b_t, p, ceng))
                for r in range(4):
                    gs.append(lambda J=J, par=par, r=r:
                              v_group(J, par, r, ceng))
                return gs

            def ag_collect():
                zgt = dp.tile([GPC * S, HPC * DH], BF16, tag="zg", name="zg")
                if collective:
                    nc.gpsimd.collective_compute(
                        "AllGather", mybir.AluOpType.bypass,
                        replica_groups=RG,
                        ins=[prtz[:].opt()], outs=[zgt[:].opt()])
                else:
                    for c in range(GPC):
                        nc.sync.dma_start(zgt[S * c:S * (c + 1), :],
                                          prtz[:])
                return zgt

            def ag_wo_a(J, zgt):
                """Load zg rows of block J + extract this rank's stripe via
                the selection-matrix matmuls (rank-independence via input)."""
                zg_sb = zgp.tile([128, GPC, 4, HPC * DH], BF16, tag="zgsb",
                                 name="zg_sb")
                for c in range(GPC):
                    base = S * c + QB * J
                    eng = nc.sync if c % 2 == 0 else nc.scalar
                    eng.dma_start(
                        zg_sb[:, c, :, :],
                        zgt[base:base + QB, :].rearrange(
                            "(k p) d -> p k d", p=128))
                tzs = []
                for c in range(GPC):
                    for h2 in range(2):
                        szT = psA.tile([128, 128], F32, tag="a_ps")
                        for k in range(4):
                            nc.tensor.matmul(
                                szT[:],
                                zg_sb[:, c, k, 128 * h2:128 * (h2 + 1)],
                                sel_t[:, k, :],
                                start=(k == 0), stop=(k == 3))
                        t_tz = ztpp.tile([128, 128], BF16, tag="ztp",
                                         name="t_tz")
                        nc.vector.tensor_copy(t_tz[:], szT[:])
                        tzs.append(t_tz)
                tz_stash[J] = tzs
                return tzs

            def ag_wo_b(J):
                tzs = tz_stash[J]
                o_sb = osbp.tile([128, D], F32, tag="o_sb")
                for n2 in range(2):
                    msl = slice(512 * n2, 512 * (n2 + 1))
                    oacc = psA.tile([128, 512], F32, tag="a_ps")
                    for kk in range(8):
                        nc.tensor.matmul(oacc[:], tzs[kk][:],
                                         wo_t[:, kk, msl],
                                         start=(kk == 0), stop=(kk == 7))
                    if bias:
                        nc.vector.tensor_tensor(o_sb[:, msl], oacc[:],
                                                bo_bc[:, msl], ADD)
                    else:
                        nc.vector.tensor_copy(o_sb[:, msl], oacc[:])
                eng = nc.sync if J % 2 == 0 else nc.scalar
                eng.dma_start(out_ext[128 * J:128 * (J + 1), :], o_sb[:])

            def wo_qsub(J, qsub, ceng=None):
                """W_O for one 128-row qsub of block J: 2 XBAR transposes +
                2 psum groups + copies + one merged prt row-block DMA."""
                for hp in range(2):
                    t_tz = ztpp.tile([128, 128], BF16, tag="ztp", name="tz")
                    eng = nc.sync if hp == 0 else nc.scalar
                    eng.dma_start_transpose(t_tz[:], zts[(J, hp, qsub)][:])
                    tz_stash[(J, qsub, hp)] = t_tz
                o_sb = osbp.tile([128, D], rdt, tag="o_sb")
                for n2 in range(2):
                    msl = slice(512 * n2, 512 * (n2 + 1))
                    oacc = psA.tile([128, 512], F32, tag="a_ps")
                    for hp in range(2):
                        nc.tensor.matmul(oacc[:], tz_stash[(J, qsub, hp)][:],
                                         wo_t[:, hp, msl],
                                         start=(hp == 0), stop=(hp == 1))
                    if bias:
                        # host pre-scales b_O by 1/GPC: every core adds
                        # bias/GPC so the ReduceScatter sum is exact.
                        nc.vector.tensor_tensor(o_sb[:, msl], oacc[:],
                                                bo_bc[:, msl], ADD)
                    else:
                        (ceng or nc.vector).tensor_copy(o_sb[:, msl], oacc[:])
                eng = nc.sync if qsub % 2 == 0 else nc.scalar
                if rs_rep:
                    eng.dma_start(
                        prtall[QB * qsub + 128 * J:QB * qsub + 128 * (J + 1),
                               :], o_sb[:])
                else:
                    eng.dma_start(
                        prt[J][128 * qsub:128 * (qsub + 1), :], o_sb[:])

            def wo_rs_out(J):
                if rs_rep:
                    if J != NJ - 1:
                        return
                    # one collective per rep over the concatenated blocks:
                    # prtall rows are (J, qsub) so the scatter slice for
                    # rank j is rows [512j:512j+512] = its stripes J=0..3
                    rs = dp.tile([QB, D], rdt, tag="rs", name="rs")
                    if collective:
                        nc.gpsimd.collective_compute(
                            "ReduceScatter", mybir.AluOpType.add,
                            replica_groups=RG,
                            ins=[prtall[:].opt()], outs=[rs[:].opt()])
                    else:
                        nc.sync.dma_start(rs[:], prtall[0:QB, :])
                    nc.scalar.dma_start(out_ext[:, :], rs[:])
                    return
                rs = dp.tile([QB // GPC, D], rdt, tag="rs", name="rs")
                if collective:
                    nc.gpsimd.collective_compute(
                        "ReduceScatter", mybir.AluOpType.add,
                        replica_groups=RG,
                        ins=[prt[J][:].opt()], outs=[rs[:].opt()])
                else:
                    nc.sync.dma_start(rs[:], prt[J][0:128, :])
                nc.scalar.dma_start(
                    out_ext[128 * J:128 * (J + 1), :], rs[:])

            def emit_S_exp(J, par, hp, c):
                qT, kT = qT_par[par], kT_par[par]
                dlt = c - 4 * J
                w0 = 128 * dlt if dlt >= 0 else 0
                csl = slice(KCH * c, KCH * (c + 1))
                qsl = slice(QB * J + w0, QB * (J + 1))
                lo, hi = slice(0, 64), slice(64, 128)
                s2 = psS.tile([KCH, 2, QB], F32, tag="s2")
                nc.tensor.matmul(s2[:, 0, w0:QB], kT[hp][lo, csl],
                                 qT[hp][lo, qsl], start=True, stop=True)
                nc.tensor.matmul(s2[:, 1, w0:QB], kT[hp][hi, csl],
                                 qT[hp][hi, qsl], start=True, stop=True)
                p2 = p2p.tile([KCH, 2, QB], BF16, tag="p2")
                nc.scalar.activation(p2[:, :, w0:QB], s2[:, :, w0:QB],
                                     EXP, scale=SCALE)
                if dlt >= 0:
                    nc.vector.tensor_mul(p2[:, :, w0:w0 + KCH],
                                         p2[:, :, w0:w0 + KCH], tri_m[:])
                return p2

            def emit_PV(J, par, hp, p2s, after_qsub=None):
                v_aug = va_par[par]
                for qsub in range(4):
                    zt = psZT.tile([KCH, 2, DH + 1], F32, tag="zt")
                    nch_q = 4 * J + qsub + 1
                    qo = 128 * qsub
                    for hh in range(2):
                        h = 2 * hp + hh
                        for c in range(nch_q):
                            nc.tensor.matmul(
                                zt[:, hh, :],
                                p2s[c][:, hh, qo:qo + 128],
                                v_aug[c][:, h, :],
                                start=(c == 0), stop=(c == nch_q - 1))
                    rsb = rsp.tile([KCH, 2, 1], F32, tag="rsb")
                    nc.vector.reciprocal(rsb[:], zt[:, :, DH:DH + 1])
                    if ag:
                        if hp == 0:
                            ztq = ztsbp.tile([KCH, 2, 2, DH], BF16,
                                             tag="ztsb", name="ztq")
                            zts[(J, 0, qsub)] = ztq
                        else:
                            ztq = zts[(J, 0, qsub)]
                        for hh in range(2):
                            nc.vector.tensor_scalar(
                                ztq[:, hp, hh, :], zt[:, hh, 0:DH],
                                rsb[:, hh, :], None, MUL)
                        if hp == 1:
                            eng = nc.sync if qsub % 2 == 0 else nc.scalar
                            eng.dma_start(
                                prtz[QB * J + 128 * qsub:
                                     QB * J + 128 * (qsub + 1), :],
                                ztq[:, :, :, :])
                    else:
                        zt_sb = ztsbp.tile([KCH, 2, DH], BF16, tag="ztsb")
                        for hh in range(2):
                            nc.vector.tensor_scalar(
                                zt_sb[:, hh, :], zt[:, hh, 0:DH],
                                rsb[:, hh, :], None, MUL)
                        zts[(J, hp, qsub)] = zt_sb
                    if after_qsub is not None:
                        after_qsub(qsub)

            deferred_next = []
            for _rep in range(reps):
                par = _rep % npar
                if ag:
                    prtz = dp.tile([S, HPC * DH], BF16, tag="prtz",
                                   name="prtz")
                if rs_rep and not ag:
                    prtall = dp.tile([GPC * QB, D], rdt, tag="prtall",
                                     name="prtall")
                for jb in range(NJ):
                    J = jb
                    nch = 4 * (J + 1)
                    prt[J] = dp.tile([QB, D], rdt, tag="prt", name="prt")
                    if _rep == 0 and J == 0:
                        # bootstrap: weights + rep-0 x/pos + QKV(0)
                        if not weights_loaded:
                            nc.sync.dma_start(wq_t[:], wqp[:, :])
                        emit_producers(par)
                        if not weights_loaded:
                            nc.scalar.dma_start(wk_t[:], wkp[:, :])
                            nc.sync.dma_start(wv_t[:], wvp[:, :])
                            if ag:
                                nc.scalar.dma_start(wo_t[:], wof[:, :])
                                nc.scalar.dma_start(sel_t[:], sel[:, :])
                            else:
                                nc.scalar.dma_start(wo_t[:], wop[:, :])
                            nc.scalar.dma_start(tri_m[:, :, :], masks[:, :])
                            nc.scalar.dma_start(bqT_t[:], bqT[:, :])
                            nc.scalar.dma_start(bkT_t[:], bkT[:, :])
                            nc.scalar.dma_start(bv_t[:], bv[:, :])
                            nc.scalar.dma_start(bo_t[:], bo[:, :])
                            weights_loaded = True
                        for g in qkv_groups(0, par):
                            g()

                    # filler: PE work injectable between score chunks
                    # fillerA: safe during the hp0 streak; fillerB: hp1.
                    # At the rep wrap (J==3), next-rep Q/K pair-p writes only
                    # conflict with this rep's hp==p streak reads, so pair-1
                    # goes to A and pair-0 to B; V(0') is deferred to B(0').
                    fillerA, fillerB = [], []
                    ag_units = []
                    if ag:
                        if zg_prev is not None:
                            zgt = zg_prev
                            ag_units = [lambda J=J, zgt=zgt: ag_wo_a(J, zgt),
                                        lambda J=J: ag_wo_b(J)]
                    elif J >= 1:
                        Jw = J - 1
                        woc = None
                        for q in range(4):
                            fillerA.append(lambda Jw=Jw, q=q, woc=woc:
                                           wo_qsub(Jw, q, woc))
                        fillerA.append(lambda Jw=Jw: wo_rs_out(Jw))
                    elif _rep >= 1:
                        for q in range(4):
                            fillerA.append(lambda q=q: wo_qsub(3, q))
                        fillerA.append(lambda: wo_rs_out(3))
                    if J == 0 and _rep >= 1:
                        fillerA = deferred_next + fillerA
                        deferred_next = []
                    pv0_hook = []
                    if J + 1 < NJ:
                        gsn = qkv_groups(J + 1, par)
                        fillerA += gsn[:4] + ag_units[:1] + gsn[4:] \
                            + ag_units[1:]
                        ag_units = []
                    elif _rep + 1 < reps:
                        adds = emit_producers(par, defer_adds=True)
                        # weave: adds one-per-unit; wo_a mid, wo_b at the end
                        wov = adds[:2] + ag_units[:1] + adds[2:] \
                            + list(fillerA)
                        fillerA.clear()
                        while wov:
                            fillerA.append(wov.pop(0))
                        fillerA += ag_units[1:]
                        ag_units = []
                        gs = qkv_groups(0, par)
                        # gs order: Q0, Q1, K0, K1, V0..V3.  Next-rep Q/K
                        # pair-0 may run during this rep's hp1 streak (its
                        # pair-0 reads are done); pair-1 only after both
                        # streaks -> injected between PV(hp0) qsubs.
                        fillerB += [gs[0], gs[2]]
                        pv0_hook = [gs[1], gs[3]]
                        deferred_next = gs[4:]

                    fillerA += ag_units
                    last_block = (_rep == reps - 1 and J == NJ - 1)
                    for hp in range(2):
                        filler = fillerA if hp == 0 else fillerB
                        if hp == 1:
                            filler += fillerA  # leftovers
                        slots = nch
                        p2s = []
                        for c in range(nch):
                            p2s.append(emit_S_exp(J, par, hp, c))
                            take = (math.ceil(len(filler) / slots)
                                    if slots > 1 else len(filler))
                            for _ in range(take):
                                filler.pop(0)()
                            slots -= 1
                        if last_block and hp == 1 and not ag:
                            # tail: fold W_O(3) into PV(3, hp1) per qsub
                            def _tail(qsub):
                                wo_qsub(3, qsub)
                                if qsub == 3:
                                    wo_rs_out(3)
                            emit_PV(J, par, hp, p2s, after_qsub=_tail)
                        elif hp == 0 and pv0_hook:
                            def _h(qsub):
                                if qsub % 2 == 1 and pv0_hook:
                                    pv0_hook.pop(0)()
                            emit_PV(J, par, hp, p2s, after_qsub=_h)
                        else:
                            emit_PV(J, par, hp, p2s)
                    if ag and J == NJ - 1:
                        zg_prev = ag_collect()
                        if _rep == reps - 1:
                            # tail: W_O for all blocks of the last rep
                            for Jt in range(NJ):
                                ag_wo_a(Jt, zg_prev)
                                ag_wo_b(Jt)
    nc.compile()
    return nc


def _make_masks():
    # [128, 2*128] causal triangle duplicated for the head-pair layout:
    # tri[k, j] = 1 if k <= j (the diagonal band of every 128-key chunk
    # relative to its causal column start)
    k = np.arange(KCH)[:, None]
    j = np.arange(KCH)[None, :]
    tri = (k <= j).astype(np.float32)
    return np.ascontiguousarray(np.concatenate([tri, tri], axis=1))


def _prechunk(w):
    """[1024, C] -> [128, DCH*C] with kc-major free layout."""
    c = w.shape[1]
    return np.ascontiguousarray(
        w.reshape(DCH, 128, c).transpose(1, 0, 2).reshape(128, DCH * c))


def make_in_maps(x, pos_embed, W_Q, b_Q, W_K, b_K, W_V, b_V, W_O, b_O,
                 ag=False):
    import ml_dtypes
    bf = ml_dtypes.bfloat16
    x = np.asarray(x, np.float32)
    pos_embed = np.asarray(pos_embed, np.float32)
    W_Q = np.asarray(W_Q, np.float32)
    W_K = np.asarray(W_K, np.float32)
    W_V = np.asarray(W_V, np.float32)
    W_O = np.asarray(W_O, np.float32)
    b_Q = np.asarray(b_Q, np.float32)
    b_K = np.asarray(b_K, np.float32)
    b_V = np.asarray(b_V, np.float32)
    b_O = np.asarray(b_O, np.float32)
    masks = _make_masks().astype(bf)
    if ag:
        wof = np.ascontiguousarray(
            W_O.reshape(8, 128, D).transpose(1, 0, 2).reshape(128, 8 * D)
        ).astype(bf)
    in_maps = []
    for c in range(N_CORES):
        g, j = divmod(c, GPC)
        hs = slice(HPC * j, HPC * (j + 1))
        wo_pair = np.ascontiguousarray(
            W_O[hs].reshape(2, 128, D).transpose(1, 0, 2).reshape(128, 2 * D))
        in_maps.append({
            "xT": np.ascontiguousarray(x[g].T).astype(bf),
            "posT": np.ascontiguousarray(pos_embed[g].T).astype(bf),
            "wqp": _prechunk(
                W_Q[hs].transpose(1, 0, 2).reshape(D, HPC * DH)).astype(bf),
            "wkp": _prechunk(
                W_K[hs].transpose(1, 0, 2).reshape(D, HPC * DH)).astype(bf),
            "wvp": _prechunk(
                W_V[hs].transpose(1, 0, 2).reshape(D, HPC * DH)).astype(bf),
            "wop": wo_pair.astype(bf),
            "bqT": np.ascontiguousarray(
                b_Q[hs].reshape(2, KCH).T).astype(np.float32),
            "bkT": np.ascontiguousarray(
                b_K[hs].reshape(2, KCH).T).astype(np.float32),
            "bv": np.ascontiguousarray(
                b_V[hs].reshape(1, HPC * DH)).astype(bf),
            "bo": np.ascontiguousarray(
                ((b_O if ag else b_O / GPC)).reshape(1, D)).astype(bf),
            "masks": masks,
        })
        if ag:
            m = in_maps[-1]
            del m["wop"]
            m["wof"] = wof
            s = np.zeros((128, GPC, 128), np.float32)
            s[np.arange(128), j, np.arange(128)] = 1.0
            m["sel"] = np.ascontiguousarray(
                s.reshape(128, GPC * 128)).astype(bf)
    return in_maps


def assemble_out(results):
    out = np.empty((B, S, D), np.float32)
    for c in range(N_CORES):
        g, j = divmod(c, GPC)
        o = results[c]["out"].astype(np.float32).reshape(NJ, 128, D)
        for J in range(NJ):
            out[g, QB * J + 128 * j:QB * J + 128 * (j + 1), :] = o[J]
    return out


_BUILT = {}


def get_built(reps: int = 1, bias: bool = True, collective: bool = True,
              rs_f32: bool = False, rs_rep: bool = False, ag: bool = False):
    key = (reps, bias, collective, rs_f32, rs_rep, ag)
    if key not in _BUILT:
        _BUILT[key] = build_nc(reps, collective=collective, bias=bias,
                               rs_f32=rs_f32, rs_rep=rs_rep, ag=ag)
    return _BUILT[key]


def kernel(**inputs) -> np.ndarray:
    use_bias = any(
        np.any(np.asarray(inputs[k])) for k in ("b_Q", "b_K", "b_V", "b_O"))
    nc = get_built(1, bias=bool(use_bias), ag=True)
    in_maps = make_in_maps(**inputs, ag=True)
    res = run_bass_kernel_spmd(nc, in_maps, list(range(N_CORES)))
    return assemble_out(res.results)
